# revision 29
# baseline (speedup 1.0000x reference)
"""MedianConvolution (gnn message passing) — Trainium2 Bass kernel, 8 cores.

Computes: h = x @ kernel; msg = h[neighbors]; out = exact midpoint median
over the K=32 neighbor axis (ranks 15,16 of the sort), i.e.
tfp percentile(q=50, interpolation='midpoint').

Distribution: nodes (rows of neighbors) are sharded across the 8 NeuronCores;
every core computes the full h = x @ kernel on-device (x/kernel replicated,
fp16 inputs, fp32 PSUM accumulate) and gathers only its own node shard's
neighbor rows.

Key layout trick: h is stored in DRAM as PAIRS h_pair[r] = [h[2r] | h[2r+1]]
(fp16, 256B rows). dma_gather requires 256B-aligned elements and int16
indices; the pair layout satisfies both (idx = node>>1 <= 24999) with ONE
descriptor per neighbor instead of the two (lo/hi halves) an fp32 layout
needs. The wrong pair half is discarded on-chip with a single
copy_predicated keyed on a host-uploaded parity mask.

The whole median datapath runs in fp16: TensorTensor min/max supports the
DVE 2x_1p fast mode (2-byte dtypes) for 2x throughput, and the final
midpoint is exact up to fp16 rounding (~0.05%), far inside the 2e-2 gate.
The x0.5 of the midpoint is folded into the GEMM weights (median is
scale-equivariant for positive scales). min/max only exist on the DVE
(the GPSIMD ucode implements only add/mult), so the whole median pipeline
lives there; plane copies go to the Scalar engine, descriptor generation
to GPSIMD, and the node shard is processed in 512-node mega-chunks (after
graduated 128/256/384-node head chunks that fill the pipeline while the
first big gathers stream in, plus a padded tail) to amortize
per-instruction overheads on the bottleneck DVE.

Per-core SPMD program:
  phase 1  GEMM: xT fp16 [256, N] x wk fp16 [256, 64] -> PSUM fp32, copied
           to fp16 and DMAed into the pair layout.
  phase 2  per chunk of C shard nodes: dma_gather pulls the 256B pair rows
           for all 32 neighbor planes (k-major, <=8192 indices per call), a
           copy_predicated resolves pair parity in place, a Batcher
           odd-even mergesort sorts planes 0-15 and 16-31 (fp16 TT min/max,
           untouched planes copied on the Scalar engine), the 32-way median
           pair comes from the anti-diagonal identity
              low = max_i min(A_i, B_15-i),  up = min_i max(A_i, B_15-i)
           via two TT ops + two min/max trees, and low+up (already scaled
           by 0.5) is written out in fp32.
"""
from contextlib import ExitStack

import numpy as np

import concourse.tile as tile
from concourse import bacc, bass_utils, library_config, mybir
from concourse.tile_rust import add_dep_helper

F32 = mybir.dt.float32
F16 = mybir.dt.float16
I16 = mybir.dt.int16
U8 = mybir.dt.uint8
P = 128
U = 64  # units
K = 32  # neighbors
FEAT = 256
NUM_CORES = 8
MEGA = 512        # main chunk size (nodes); tail chunk is the padded rest
NET_BUFS = 2
MAX_GATHER = 8192  # max indices per dma_gather call (HW limit: 16384 fails)

# Batcher odd-even mergesort(16) stages; verified against np.sort via the
# 0-1 principle. Each stage: comparators (k, k+d) for k = i*f + r over the
# slices below, applied to both 16-plane halves. cp = untouched plane
# slices (copied to the ping-pong destination).
SORT16_STAGES = [
    dict(f=2, i=(0, 8, 1), r=(0, 1, 1), d=1, cp=[]),
    dict(f=4, i=(0, 4, 1), r=(0, 2, 1), d=2, cp=[]),
    dict(f=4, i=(0, 4, 1), r=(1, 2, 1), d=1, cp=[(0, 16, 4), (3, 16, 4)]),
    dict(f=8, i=(0, 2, 1), r=(0, 4, 1), d=4, cp=[]),
    dict(f=8, i=(0, 2, 1), r=(2, 4, 1), d=2,
         cp=[(0, 16, 8), (1, 16, 8), (6, 16, 8), (7, 16, 8)]),
    dict(f=8, i=(0, 2, 1), r=(1, 6, 2), d=1, cp=[(0, 16, 8), (7, 16, 8)]),
    dict(f=16, i=(0, 1, 1), r=(0, 8, 1), d=8, cp=[]),
    dict(f=16, i=(0, 1, 1), r=(4, 8, 1), d=4, cp=[(0, 4, 1), (12, 16, 1)]),
    dict(f=4, i=(0, 3, 1), r=(2, 4, 1), d=2, cp=[(0, 2, 1), (14, 16, 1)]),
    dict(f=2, i=(0, 7, 1), r=(1, 2, 1), d=1, cp=[(0, 16, 15)]),
]


def _chunk_grid(shard):
    """Chunk sizes covering `shard` nodes: graduated small head chunks (so
    the sort pipeline fills while the first big gathers stream in after the
    GEMM), MEGA-chunks for the bulk, and a padded tail."""
    grid = []
    rest = shard
    for head in (P, 2 * P, 3 * P):
        if rest > head + MEGA:
            grid.append(head)
            rest -= head
    grid += [MEGA] * (rest // MEGA)
    rem = rest - (rest // MEGA) * MEGA
    if rem:
        grid.append(((rem + P - 1) // P) * P)
    return grid


def build_kernel(nrows, shard, num_cores=NUM_CORES, gemm_super=3072,
                 net_bufs=NET_BUFS, head_thr=None):
    """head_thr: optional per-head-chunk tuples of 4 supertile counts, one
    per 8-plane neighbor group (host-computed from the actual neighbor
    lists): group g of head chunk c only needs the first head_thr[c][g]
    h supertiles, so its gather and sort8 overlap the GEMM's tail."""
    # pad to a multiple of 2*P: every partition then owns an even number of
    # GEMM tiles, so host column-pairing (see _prep_inputs) makes each
    # partition's h pair rows contiguous in DRAM
    nrows = ((nrows + 2 * P - 1) // (2 * P)) * (2 * P)
    NPAIR = nrows // 2
    grid = _chunk_grid(shard)
    IDXTOT = sum(Cc * K // 16 for Cc in grid)
    JTOT = sum(Cc * K // P for Cc in grid)
    BUTOT = sum(Cc // 2 for Cc in grid)

    nc = bacc.Bacc(
        "TRN2",
        target_bir_lowering=False,
        debug=False,
        num_devices=num_cores,
    )

    xT = nc.dram_tensor("xT", [FEAT, nrows], F16, kind="ExternalInput").ap()
    wk = nc.dram_tensor("wk", [FEAT, U], F16, kind="ExternalInput").ap()
    idx = nc.dram_tensor("idx", [P, IDXTOT], I16, kind="ExternalInput").ap()
    par = nc.dram_tensor("par", [P, JTOT], U8, kind="ExternalInput").ap()
    out = nc.dram_tensor("out", [P, BUTOT], F32, kind="ExternalOutput").ap()
    h_pair = nc.dram_tensor("h_pair", [NPAIR, 2 * U], F16, kind="Internal").ap()

    # manual progress semaphore for the h writes: the tile scheduler rounds
    # instruction-level deps up to coarse engine-progress waits (measured:
    # a dep on h supertiles 0..9 became "Activation >= tick of supertile
    # ~15"), so the partial-h head gathers gate on this sem instead. Each
    # h-write DMA incs it by 16 (the HWDGE completion convention).
    hsem = nc.alloc_semaphore(name="h_prefix") if head_thr else None

    with tile.TileContext(nc) as tc:
        with ExitStack() as ctx:
            # ---------------- phase 1: GEMM ----------------
            ctx1 = ctx.enter_context(ExitStack())
            g_x = ctx1.enter_context(tc.tile_pool(name="g_x", bufs=3))
            g_w = ctx1.enter_context(tc.tile_pool(name="g_w", bufs=1))
            g_h = ctx1.enter_context(tc.tile_pool(name="g_h", bufs=3))
            g_ps = ctx1.enter_context(tc.tile_pool(name="g_ps", bufs=8, space="PSUM"))

            wkt = g_w.tile([P, 2 * U], F16)
            nc.sync.dma_start(wkt[:, 0:U], wk[0:P, :])
            nc.sync.dma_start(wkt[:, U : 2 * U], wk[P : 2 * P, :])

            h_writes = []
            S = gemm_super
            n_super = (nrows + S - 1) // S
            for s in range(n_super):
                n0 = s * S
                ncnt = min(S, nrows - n0)
                assert ncnt % P == 0
                ntiles = ncnt // P
                xt0 = g_x.tile([P, S], F16, tag="xt0")
                xt1 = g_x.tile([P, S], F16, tag="xt1")
                nc.sync.dma_start(xt0[:, 0:ncnt], xT[0:P, n0 : n0 + ncnt])
                nc.sync.dma_start(xt1[:, 0:ncnt], xT[P : 2 * P, n0 : n0 + ncnt])
                hb = g_h.tile([P, (S // P) * U], F16, tag="hb")
                for t in range(0, ntiles, 2):
                    # two node-tiles share one PSUM tile so each Activation
                    # copy moves 128 cols: the per-op overhead of the
                    # PSUM->SBUF copies paces the GEMM phase otherwise
                    npair = min(2, ntiles - t)
                    ps = g_ps.tile([P, 2 * U], F32)
                    for q in range(npair):
                        c0 = (t + q) * P
                        nc.tensor.matmul(
                            ps[:, q * U : (q + 1) * U],
                            xt0[:, c0 : c0 + P], wkt[:, 0:U],
                            start=True, stop=False,
                        )
                        nc.tensor.matmul(
                            ps[:, q * U : (q + 1) * U],
                            xt1[:, c0 : c0 + P], wkt[:, U : 2 * U],
                            start=False, stop=True,
                        )
                    nc.scalar.copy(
                        hb[:, t * U : (t + npair) * U], ps[:, 0 : npair * U]
                    )
                # pair layout: node n -> pair row n>>1, half n&1. The host
                # orders xT columns so GEMM tile t / partition p computes
                # node 2*(p*T_h + t//2) + (t&1): both halves of a pair land
                # in ONE partition at adjacent tile slots, and partition p's
                # pair rows are consecutive in DRAM -- the h write is one
                # contiguous multi-KB run per partition instead of 50k
                # strided 256B descriptors (which dominated the startup).
                assert ntiles % 2 == 0
                T_h = nrows // (2 * P)
                th_off = (n0 // P) // 2
                dr = h_pair[:].rearrange("(p th) su -> p (th su)", p=P)[
                    :, th_off * 2 * U : (th_off + ntiles // 2) * 2 * U
                ]
                # issue the h write from the Activation queue (which just
                # produced hb): a sync-engine DMA would make SP block on the
                # GEMM-copy semaphore and pace the x loads at h-write rate
                hw = nc.scalar.dma_start(dr, hb[:, 0 : ntiles * U])
                if hsem is not None:
                    hw.then_inc(hsem, 16)
                h_writes.append(hw)

            # ---------------- phase 2: gather + median ----------------
            ctx1.close()
            g_net = ctx.enter_context(tc.tile_pool(name="g_net", bufs=net_bufs))
            g_srt = ctx.enter_context(tc.tile_pool(name="g_srt", bufs=2))
            g_idx = ctx.enter_context(tc.tile_pool(name="g_idx", bufs=4))
            g_out = ctx.enter_context(tc.tile_pool(name="g_out", bufs=2))
            g_cmp = ctx.enter_context(tc.tile_pool(name="g_cmp", bufs=1))

            nc.gpsimd.load_library(library_config.mlp)
            # all per-chunk tiles are allocated at MEGA size and prefix-
            # sliced for smaller chunks, so every chunk shares one set of
            # double-buffered tags
            BM = MEGA // P
            state = dict(full_dep=False)
            offs = []
            ioff = joff = ooff = 0
            for C in grid:
                offs.append((ioff, joff, ooff))
                ioff += C * K // 16
                joff += C * K // P
                ooff += C // 2

            def emit_gather(c):
                """idx/par loads + the dma_gather calls for chunk c. Emitted
                one chunk ahead of the sort so the gather preps are not
                queued behind the previous chunk's GPSIMD stage-1 ops."""
                C = grid[c]
                NIDX = C * K
                IDXCOLS = NIDX // 16
                J = NIDX // P
                ioff_c, joff_c, _ = offs[c]
                ia_f = g_idx.tile([P, MEGA * K // 16], I16, tag="ia")
                pa_f = g_idx.tile([P, MEGA * K // P], U8, tag="pa")
                ia = ia_f[:, 0:IDXCOLS]
                pa = pa_f[:, 0:J]
                n_head = len(head_thr) if head_thr is not None else 0
                # head-chunk idx/par loads issue from the (phase-1-idle)
                # GPSIMD queue: on SP they would sit behind all 34 x loads
                # (SP blocks on each x load's tile-reuse semaphore), gating
                # the first gather on the whole x stream
                idxeng = nc.gpsimd if c < n_head else nc.sync
                idxeng.dma_start(ia, idx[:, ioff_c : ioff_c + IDXCOLS])
                idxeng.dma_start(pa, par[:, joff_c : joff_c + J])
                pt_f = g_net.tile([P, (MEGA * K // P) * 2 * U], F16, tag="pt")
                pt = pt_f[:, 0 : J * 2 * U]
                # gather calls of <=8192 256B pair rows each; early chunks
                # are split finer so parity resolution starts before each
                # whole gather lands (shortens the phase-1 ramp)
                split = max(1, NIDX // MAX_GATHER)
                if c < n_head:
                    split = 4  # one call per 8-plane neighbor group
                elif c < 3:
                    split *= 2
                per = NIDX // split
                assert per % P == 0
                for gsp in range(split):
                    if c < n_head:
                        # gate on the h-prefix this group actually reads
                        # (neighbors are host-sorted by h write time)
                        nc.gpsimd.wait_ge(hsem, 16 * head_thr[c][gsp])
                    g = nc.gpsimd.dma_gather(
                        pt[:, gsp * per // P * 2 * U : (gsp + 1) * per // P * 2 * U]
                        .rearrange("p (j e) -> p j e", e=2 * U),
                        h_pair[:],
                        ia[:, gsp * per // 16 : (gsp + 1) * per // 16],
                        per,
                        per,
                        2 * U,
                        single_packet=False,
                    )
                    if c < n_head:
                        pass  # gated by the wait_ge above
                    elif not state["full_dep"]:
                        for w in h_writes:
                            add_dep_helper(
                                g.ins, w.ins,
                                reason="gather waits for h DRAM writes",
                            )
                        state["full_dep"] = True
                return dict(pt=pt, pa=pa, split=split, per=per, J=J)

            def emit_tail(c, vlu, BU, ooff):
                """Min/max trees + midpoint + out DMA for chunk c. Emitted
                AFTER chunk c+1's sort stages so the DVE never head-of-line
                blocks on the GPSIMD antidiag chain of chunk c."""
                w = 16
                while w > 1:
                    h = w // 2
                    nc.vector.tensor_tensor(
                        out=vlu[:, 0, 0:h, :], in0=vlu[:, 0, 0:h, :],
                        in1=vlu[:, 0, h:w, :], op=mybir.AluOpType.max,
                    )
                    nc.vector.tensor_tensor(
                        out=vlu[:, 1, 0:h, :], in0=vlu[:, 1, 0:h, :],
                        in1=vlu[:, 1, h:w, :], op=mybir.AluOpType.min,
                    )
                    w = h
                med_f = g_out.tile([P, BM * U], F32, tag="med")
                med = med_f[:, 0:BU]
                # fp16+fp16->fp32 add on the Pool ucode keeps the last op of
                # each chunk off the bottleneck DVE (the final chunk stays on
                # DVE: nothing left to overlap, and skipping the Q7 launch
                # shortens the drain)
                addeng = nc.gpsimd if c < len(grid) - 1 else nc.vector
                addeng.tensor_tensor(
                    out=med, in0=vlu[:, 0, 0, :], in1=vlu[:, 1, 0, :],
                    op=mybir.AluOpType.add,
                )
                nc.sync.dma_start(out[:, ooff : ooff + BU], med)

            # gathers run TWO chunks ahead: the GPSIMD antidiag of chunk c
            # head-of-line blocks the Pool queue while the DVE sorts chunk
            # c, so the prep for chunk c+2's gather must already be queued
            # in front of it for the DMA to land before the sort needs it
            gq = [emit_gather(0)]
            if len(grid) > 1:
                gq.append(emit_gather(1))
            tail_pending = None
            for c, C in enumerate(grid):
                B = C // P
                BU = B * U
                J = C * K // P
                _, _, ooff = offs[c]
                gi = gq.pop(0)
                pt, pa = gi["pt"], gi["pa"]
                if c + 2 < len(grid):
                    gq.append(emit_gather(c + 2))
                # parity select runs on an int32 bitcast view: the mask is
                # uniform across a slot's 64 units, and CopyPredicated has
                # no fast modes, so packing 2 fp16 per element halves its
                # per-element cost
                p4i = pt.bitcast(mybir.dt.int32).rearrange(
                    "p (j s w) -> p j s w", s=2, w=U // 2
                )

                def emit_parity(jj0, jj1):
                    # resolve pair parity in place: even half (s=0) is the
                    # message home; overwrite with odd half where parity=1
                    mask = pa[:, jj0:jj1].unsqueeze(2).broadcast_to(
                        [P, jj1 - jj0, U // 2]
                    )
                    nc.vector.copy_predicated(
                        out=p4i[:, jj0:jj1, 0, :], mask=mask,
                        data=p4i[:, jj0:jj1, 1, :],
                    )

                ra_f = g_srt.tile([P, K * BM * U], F16, tag="ra")
                rb_f = g_srt.tile([P, K * BM * U], F16, tag="rb")
                ra = ra_f[:, 0 : K * BU]
                rb = rb_f[:, 0 : K * BU]

                # stage 1 of the Batcher network reads the strided message
                # view (s=0 halves of pt) and writes compact k-major planes
                msg = pt.rearrange(
                    "p (hi r b s u) -> p hi r b s u", hi=16, r=2, b=B, s=2, u=U
                )[:, :, :, :, 0, :]

                def emit_stage1(hi0, hi1):
                    vd = ra.rearrange("p (hi r bu) -> p hi r bu", r=2, bu=BU)
                    lo_s = msg[:, hi0:hi1, 0, :, :]
                    hi_s = msg[:, hi0:hi1, 1, :, :]
                    nc.vector.tensor_tensor(
                        out=vd[:, hi0:hi1, 0, :].rearrange(
                            "p hi (b u) -> p hi b u", u=U
                        ),
                        in0=lo_s, in1=hi_s, op=mybir.AluOpType.min,
                    )
                    nc.vector.tensor_tensor(
                        out=vd[:, hi0:hi1, 1, :].rearrange(
                            "p hi (b u) -> p hi b u", u=U
                        ),
                        in0=lo_s, in1=hi_s, op=mybir.AluOpType.max,
                    )

                def stage_geom(sp):
                    f = sp["f"]
                    d = sp["d"]
                    di, dr = d // f, d % f
                    r_vals = list(range(*sp["r"]))
                    if r_vals[-1] + dr >= f:
                        assert all(rv + dr >= f for rv in r_vals), sp
                        di, dr = di + 1, dr - f
                    r_sl = slice(*sp["r"])
                    hi_r = slice(sp["r"][0] + dr, sp["r"][1] + dr, sp["r"][2])
                    return f, 16 // f, di, r_sl, hi_r

                def emit_stage(sp, src, dst):
                    """One Batcher stage over both halves (full width)."""
                    f, ni, di, r_sl, hi_r = stage_geom(sp)
                    i_full = sp["i"] == (0, ni, 1)
                    if i_full and di == 0:
                        vs = src.rearrange("p (hi r bu) -> p hi r bu", r=f, bu=BU)
                        vd = dst.rearrange("p (hi r bu) -> p hi r bu", r=f, bu=BU)
                        lo_s = vs[:, :, r_sl, :]
                        hi_s = vs[:, :, hi_r, :]
                        nc.vector.tensor_tensor(
                            out=vd[:, :, r_sl, :], in0=lo_s, in1=hi_s,
                            op=mybir.AluOpType.min,
                        )
                        nc.vector.tensor_tensor(
                            out=vd[:, :, hi_r, :], in0=lo_s, in1=hi_s,
                            op=mybir.AluOpType.max,
                        )
                    else:
                        i_sl = slice(*sp["i"])
                        hi_i = slice(sp["i"][0] + di, sp["i"][1] + di, sp["i"][2])
                        vs = src.rearrange(
                            "p (hh i r bu) -> p hh i r bu", hh=2, i=ni, r=f, bu=BU
                        )
                        vd = dst.rearrange(
                            "p (hh i r bu) -> p hh i r bu", hh=2, i=ni, r=f, bu=BU
                        )
                        lo_s = vs[:, :, i_sl, r_sl, :]
                        hi_s = vs[:, :, hi_i, hi_r, :]
                        nc.vector.tensor_tensor(
                            out=vd[:, :, i_sl, r_sl, :], in0=lo_s, in1=hi_s,
                            op=mybir.AluOpType.min,
                        )
                        nc.vector.tensor_tensor(
                            out=vd[:, :, hi_i, hi_r, :], in0=lo_s, in1=hi_s,
                            op=mybir.AluOpType.max,
                        )
                    vks = src.rearrange("p (hh kk bu) -> p hh kk bu", hh=2, kk=16)
                    vkd = dst.rearrange("p (hh kk bu) -> p hh kk bu", hh=2, kk=16)
                    for cpsl in sp["cp"]:
                        ks = slice(*cpsl)
                        nc.scalar.copy(vkd[:, :, ks, :], vks[:, :, ks, :])

                def emit_stage_group(sp, src, dst, hh, m):
                    """One of stages s1..s5 restricted to the 8-plane group
                    [8m, 8m+8) of half hh (those stages are two independent
                    sort8s per half, so the restriction is exact)."""
                    f, ni, di, r_sl, hi_r = stage_geom(sp)
                    assert di == 0, sp
                    i0 = max(sp["i"][0], 8 * m // f)
                    i1 = min(sp["i"][1], (8 * m + 8) // f)
                    assert sp["i"][2] == 1
                    vs = src.rearrange(
                        "p (hh i r bu) -> p hh i r bu", hh=2, i=ni, r=f, bu=BU
                    )
                    vd = dst.rearrange(
                        "p (hh i r bu) -> p hh i r bu", hh=2, i=ni, r=f, bu=BU
                    )
                    lo_s = vs[:, hh : hh + 1, i0:i1, r_sl, :]
                    hi_s = vs[:, hh : hh + 1, i0:i1, hi_r, :]
                    nc.vector.tensor_tensor(
                        out=vd[:, hh : hh + 1, i0:i1, r_sl, :],
                        in0=lo_s, in1=hi_s, op=mybir.AluOpType.min,
                    )
                    nc.vector.tensor_tensor(
                        out=vd[:, hh : hh + 1, i0:i1, hi_r, :],
                        in0=lo_s, in1=hi_s, op=mybir.AluOpType.max,
                    )
                    vks = src.rearrange("p (hh kk bu) -> p hh kk bu", hh=2, kk=16)
                    vkd = dst.rearrange("p (hh kk bu) -> p hh kk bu", hh=2, kk=16)
                    for st, en, step in sp["cp"]:
                        vals = [v for v in range(st, en, step)
                                if 8 * m <= v < 8 * m + 8]
                        if vals:
                            ks = slice(vals[0], vals[-1] + 1, step)
                            nc.scalar.copy(
                                vkd[:, hh : hh + 1, ks, :],
                                vks[:, hh : hh + 1, ks, :],
                            )

                n_head = len(head_thr) if head_thr is not None else 0
                if c < n_head:
                    # per 8-plane group: parity + stage 1 + the sort8
                    # (stages s1..s5) as soon as that group's gather lands
                    for g in range(4):
                        emit_parity(g * (J // 4), (g + 1) * (J // 4))
                        emit_stage1(4 * g, 4 * g + 4)
                        src, dst = ra, rb
                        for sp in SORT16_STAGES[1:6]:
                            emit_stage_group(sp, src, dst, g // 2, g % 2)
                            src, dst = dst, src
                    # merge stages need all planes of a half
                    src, dst = (ra, rb) if len(SORT16_STAGES[1:6]) % 2 == 0 \
                        else (rb, ra)
                    for sp in SORT16_STAGES[6:]:
                        emit_stage(sp, src, dst)
                        src, dst = dst, src
                else:
                    for gsp in range(gi["split"]):
                        emit_parity(gsp * (gi["per"] // P),
                                    (gsp + 1) * (gi["per"] // P))
                    emit_stage1(0, 16)
                    src, dst = ra, rb
                    for sp in SORT16_STAGES[1:]:
                        emit_stage(sp, src, dst)
                        src, dst = dst, src

                # anti-diagonal merge of the two sorted 16-plane halves as a
                # relu-comparator on GPSIMD+Scalar (the Pool ucode only has
                # add/mult): lo = A - relu(A-Brev), up = Brev + relu(A-Brev).
                # Takes ~4.4us/chunk off the bottleneck DVE; off the
                # inter-chunk critical path because only the trees consume
                # it, and those are emitted a chunk later. Exact up to one
                # fp16 ulp of (A-Brev). Results go to a separate small tile
                # so ra/rb free up once the antidiag retires.
                vk = src.rearrange("p (k bu) -> p k bu", k=K)
                A = vk[:, 0:16, :]
                Brev = vk[:, 31:15:-1, :]
                tlu_f = g_out.tile([P, 2 * 16 * BM * U], F16, tag="tlu")
                vlu = tlu_f[:, 0 : 2 * 16 * BU].rearrange(
                    "p (s k bu) -> p s k bu", s=2, bu=BU
                )
                nc.vector.tensor_tensor(
                    out=vlu[:, 0, :, :], in0=A, in1=Brev, op=mybir.AluOpType.min
                )
                nc.vector.tensor_tensor(
                    out=vlu[:, 1, :, :], in0=A, in1=Brev, op=mybir.AluOpType.max
                )
                if tail_pending is not None:
                    emit_tail(*tail_pending)
                tail_pending = (c, vlu, BU, ooff)
            emit_tail(*tail_pending)

    nc.compile()
    return nc


def _prep_inputs(x, neighbors, kern, num_cores=NUM_CORES):
    nrows = x.shape[0]
    total = neighbors.shape[0]
    shard = (total + num_cores - 1) // num_cores
    grid = _chunk_grid(shard)
    shard_pad = sum(grid)
    IDXTOT = sum(Cc * K // 16 for Cc in grid)
    JTOT = sum(Cc * K // P for Cc in grid)

    nrows_pad = ((nrows + 2 * P - 1) // (2 * P)) * (2 * P)
    # xT column (t_glob*128 + p) computes node 2*(p*T_h + t_glob//2) +
    # (t_glob&1): see the h-write comment in build_kernel. The DRAM pair
    # layout stays node>>1, so the neighbor index prep is unchanged.
    T_h = nrows_pad // (2 * P)
    cols = np.arange(nrows_pad)
    t_glob = cols // P
    p_col = cols % P
    node_of_col = 2 * (p_col * T_h + t_glob // 2) + (t_glob & 1)
    xT = np.zeros((FEAT, nrows_pad), dtype=np.float16)
    valid = node_of_col < nrows
    xT[:, valid] = x.T[:, node_of_col[valid]]
    # fold the midpoint *0.5 into the weights (median is scale-equivariant)
    wk = np.ascontiguousarray(kern * 0.5).astype(np.float16)

    n_super = (nrows_pad + 3071) // 3072
    pairs_per_super = 3072 // (2 * P)
    N_HEAD = min(3, max(0, len(grid) - 1))
    head_thr = [[1] * 4 for _ in range(N_HEAD)]
    in_maps = []
    for core in range(num_cores):
        n0 = core * shard
        nbr = np.zeros((shard_pad, K), dtype=np.int64)
        real = min(shard, total - n0)
        nbr[:real] = neighbors[n0 : n0 + real]
        # sort each node's neighbors by h-write time (supertile s writes
        # pair rows with r % T_h in [12s, 12s+12)): the first 8-plane
        # groups of the head chunks then only need a prefix of the h
        # supertiles, letting their gathers overlap the GEMM tail. The
        # median is invariant under per-node neighbor permutation.
        key = (nbr >> 1) % T_h
        order = np.argsort(key, axis=1, kind="stable")
        nbr = np.take_along_axis(nbr, order, axis=1)
        key = np.take_along_axis(key, order, axis=1)
        idxarr = np.empty((P, IDXTOT), dtype=np.int16)
        pararr = np.empty((P, JTOT), dtype=np.uint8)
        noff = ioff = joff = 0
        for ci, Cc in enumerate(grid):
            if ci < N_HEAD:
                kc = key[noff : noff + Cc]
                for g in range(4):
                    maxth = int(kc[:, 8 * g + 7].max())
                    sg = min(n_super, maxth // pairs_per_super + 1)
                    head_thr[ci][g] = max(head_thr[ci][g], sg)
            B = Cc // P
            NIDX = Cc * K
            IDXCOLS = NIDX // 16
            J = NIDX // P
            # i = ((k*B + b)*128 + p) enumerates (plane k, block b, part p)
            nb = (
                nbr[noff : noff + Cc]
                .reshape(B, P, K)
                .transpose(2, 0, 1)
                .reshape(NIDX)
            )
            pairs = (nb >> 1).astype(np.int16)
            parity = (nb & 1).astype(np.uint8)
            # logical index i lives at [i%16, i//16]; replicated to all
            # eight 16-partition groups (Q7 core pairs read their own)
            idxarr[:, ioff : ioff + IDXCOLS] = np.tile(
                pairs.reshape(IDXCOLS, 16).T, (P // 16, 1)
            )
            pararr[:, joff : joff + J] = parity.reshape(J, P).T
            noff += Cc
            ioff += IDXCOLS
            joff += J
        in_maps.append({"xT": xT, "wk": wk, "idx": idxarr, "par": pararr})
    meta = dict(shard=shard, shard_pad=shard_pad, grid=grid, total=total,
                head_thr=tuple(tuple(t) for t in head_thr))
    return in_maps, meta


def _unshard_output(results, meta, num_cores=NUM_CORES):
    outs = []
    for core in range(num_cores):
        o = results[core]["out"]  # [P, BUTOT]
        parts = []
        ooff = 0
        for Cc in meta["grid"]:
            B = Cc // P
            BU = B * U
            blk = o[:, ooff : ooff + BU].reshape(P, B, U).transpose(1, 0, 2)
            parts.append(blk.reshape(Cc, U))
            ooff += BU
        outs.append(np.concatenate(parts, axis=0)[: meta["shard"]])
    return np.concatenate(outs, axis=0)[: meta["total"]]


_CACHE = {}


def kernel(x, neighbors, kernel):
    """Full inputs in, full output out. Shards nodes across 8 NeuronCores."""
    x = np.asarray(x, dtype=np.float32)
    neighbors_np = np.asarray(neighbors)
    kern = np.asarray(kernel, dtype=np.float32)
    assert x.shape[1] == FEAT and kern.shape == (FEAT, U)
    assert neighbors_np.shape[1] == K

    in_maps, meta = _prep_inputs(x, neighbors_np, kern)
    key = (x.shape[0], meta["shard"], meta["head_thr"])
    if key not in _CACHE:
        _CACHE[key] = build_kernel(
            x.shape[0], meta["shard"], head_thr=meta["head_thr"]
        )
    nc = _CACHE[key]
    res = bass_utils.run_bass_kernel_spmd(
        nc, in_maps, core_ids=list(range(NUM_CORES))
    )
    return _unshard_output(res.results, meta)

